# revision 21
# baseline (speedup 1.0000x reference)
"""LMS adaptive filter (BaseFilter) on 8 TRN2 NeuronCores.

Algorithm per (batch b, frame f): 64-tap LMS over 416 sequential steps.
  e_t   = d[b, 256f + 32 + t] - sum_k w[k] * x[256f + t + k]
  w     = clip(w + MU * e_t * x[256f + t : +64], +-65535)
The clip is essential: mu*|x_win|^2 ~ 3.2 > 2 makes the recursion
unstable, so w rides the clip rails and the rails keep all float
implementations shadowing each other.
Outputs (overlap-add): d_est = d - e and e, assembled per reference.

Sharding: 4096 frames split 512/core (both batches on every core) ->
1024 independent sequences/core = 2 chains (one per batch) x 4
frame-groups x 128 partitions.

Per step, per chain, ops are WIDE (all 4 groups at once, 256 elems):
  DVE   custom scan-dot-clip : SC = cumsum(-(clip(V) * xs_win)) run
        continuously across the 4 groups; group g's x is pre-scaled by
        2^(30g) (exact in fp32), which puts each group's dot in a
        disjoint exponent range so differencing the boundary columns
        recovers each per-group dot at full local precision (plain
        differencing of an unscaled cumsum loses ~1e-2 rel; scaled
        differencing measures 8e-5 in numpy vs the reference).
  GpSimd tensor_tensor x2    : dif = (d_scaled_t + cum_g) - cum_{g-1}
        (= 2^(30g) * e_t, stored to EB; descaled once at the end)
  GpSimd tensor_tensor       : U = dif_bcast * (MU * 2^-30g * x_win)
        (stride-0 broadcast AP; the descale is folded into the
        precomputed multiplier tile, so U = MU * e * x exactly)
  DVE   custom add-clip      : V = clip(V) + U   (V stored unclipped;
        clip applied inline on every read -> no separate clip pass)

Scheduling is as important as the op set: the Tile list-scheduler
otherwise collapses the two chains into lockstep (both Us serialized
on GpSimd while DVE idles). Three measures force a software-pipelined
antiphase steady state: (1) chain 1's V is zero-initialized THROUGH
chain 0's first U product, so chain 1 starts half a step behind;
(2) each add-clip carries a nosync (order-only) dependency on the
OTHER chain's just-emitted dif, which pins the per-period DVE order;
(3) U is double-buffered by step parity so GpSimd's U(t+1) write does
not wait on the clip(t) read. Steady state: GpSimd ~2.5us/step busy
(U 794ns, T1 200ns, dif 270ns per chain) vs DVE ~1.8us (customs
~424ns each), period ~2.88us/step.

Measured alternatives that LOST: scalar_tensor_tensor is rejected on
the GpSimd engine by neuronxcc; Scalar ACTIVATE (scale=e_col) costs
513ns per 64-elem group (222-cycle SBUF access); narrow per-group STT
updates (baseline-style, no U tile) cost ~200ns effective here vs
119ns in the pure-narrow baseline; putting T1/dif on DVE exposes
~300ns write-ack stalls between a chain's clip and its next scan;
dif-on-DVE alone adds GpS->DVE->GpS sem ping-pong (1707us); an
EB layout transposed to [128, S, G] for contiguous dif writes fell
off the GpSimd fast path entirely (2209us). bf16 anywhere diverges:
the clipped-unstable recursion amplifies perturbations ~1e3x.
"""

import numpy as np

HOP = 256
FRAMELEN = 512
K = 64
WD = 32
MU = 0.05
WMIN, WMAX = -65535.0, 65535.0
B = 2
F = 4096
NC = 8
F_LOC = F // NC              # 512 frames per core
G = 4                        # frame-groups per chain (1 chain per batch)
SCALE_BITS = 30              # per-group exponent separation
S = (FRAMELEN - K) - WD      # 416 sequential steps
TSTART = (FRAMELEN - HOP) - WD  # 224: first step kept for frames >= 1
TAIL = S - TSTART            # 192 output elements per frame >= 1
SPAN = HOP * (F_LOC - 1) + FRAMELEN  # 131328: x/d elements per core shard
CORE_STRIDE = HOP * F_LOC    # 131072
OUT_LEN = (FRAMELEN - K) + (F - 1) * TAIL  # 786688

_CACHE = {}


def _custom_ops():
    """Register the two fused DVE ops (runtime registration; shas computed
    live so the pinned-sha check in dve_table_for_ops passes)."""
    import concourse.dve_ops as dve_ops
    from concourse.dve_ops import DveOp
    from concourse.dve_spec import (
        Spec, Src0, Src1, C0, C1, Zero, scan, AluOp as DveAluOp,
        minn, maxx, lower, _has_src1,
    )
    from concourse.dve_uop import DveOpSpec

    def _ref_scandotclip(in0, in1, s0, s1, imm2):
        p = in0.shape[0]
        a = in0.astype(np.float32).reshape(p, -1)
        b = in1.astype(np.float32).reshape(p, -1)
        w = np.clip(a, np.asarray(s1, np.float32).reshape(-1, 1),
                    np.asarray(s0, np.float32).reshape(-1, 1))
        return np.cumsum(-(w * b), axis=-1).astype(np.float32)

    def _ref_addclip(in0, in1, s0, s1, imm2):
        w = np.clip(in0.astype(np.float32),
                    np.asarray(s1, np.float32).reshape(-1, 1),
                    np.asarray(s0, np.float32).reshape(-1, 1))
        return (w + in1.astype(np.float32)).astype(np.float32)

    def _register(name, spec, subdim=False):
        for op in dve_ops.OPS:
            if op.name == name:
                return op
        shas = {}
        for ver in ("v3", "v4"):
            tmp = DveOpSpec(name=name, opcode=1, uops=lower(spec, ver=ver),
                            rd1_en=_has_src1(spec))
            shas[ver] = tmp.sha(ver)
        op = DveOp(name, spec, subdim=subdim, uops_sha=shas)
        dve_ops.OPS.append(op)
        dve_ops.CUSTOM_DVE_SPECS[name] = spec
        dve_ops._SUB_OPCODE_FOR_NAME[name] = (
            dve_ops._CUSTOM_DVE_ROW_BASE + len(dve_ops.OPS) - 1)
        return op

    scandotclip = _register(
        "LMS_SCANDOT_CLIP",
        Spec(body=scan(DveAluOp.ADD,
                       Zero - maxx(minn(Src0, C0), C1) * Src1, init=Zero),
             reference=_ref_scandotclip),
    )
    addclip = _register(
        "LMS_ADD_CLIP",
        Spec(body=maxx(minn(Src0, C0), C1) + Src1,
             reference=_ref_addclip),
    )
    return scandotclip, addclip


def _build():
    import concourse.bacc as bacc
    import concourse.tile as tile
    from concourse import mybir
    import concourse.bass as bass

    f32 = mybir.dt.float32
    AluOp = mybir.AluOpType
    Act = mybir.ActivationFunctionType
    scandotclip, addclip = _custom_ops()

    nc = bacc.Bacc("TRN2", target_bir_lowering=False)
    x_in = nc.dram_tensor("x", [SPAN], f32, kind="ExternalInput")
    d_in = nc.dram_tensor("d", [B, SPAN], f32, kind="ExternalInput")
    # [kind(0=d_est,1=e)][b][f_local][j] , j <-> step t = TSTART + j
    out_main = nc.dram_tensor("out_main", [2, B, F_LOC, TAIL], f32,
                              kind="ExternalOutput")
    # frame 0 of this core: steps t < TSTART   [kind][b][t]
    out_head = nc.dram_tensor("out_head", [2, B, TSTART], f32,
                              kind="ExternalOutput")

    def sub_ap(base, offset_add, free_dims):
        # raw AP into a tile: keep partition dim, replace free dims
        return bass.AP(tensor=base.tensor, offset=base.offset + offset_add,
                       ap=[base.ap[0]] + free_dims)

    sg = [float(2.0 ** (SCALE_BITS * g)) for g in range(G)]
    sginv = [float(2.0 ** (-SCALE_BITS * g)) for g in range(G)]

    with tile.TileContext(nc) as tc:
        with tc.tile_pool(name="p", bufs=1) as pool:
            XF = pool.tile([128, G, FRAMELEN], f32)    # x frames (slab fg)
            XFS = pool.tile([128, G, FRAMELEN], f32)   # 2^(30g) * x
            XFMUS = pool.tile([128, G, FRAMELEN], f32)  # MU * 2^-30g * x
            DB = pool.tile([128, B, G, S], f32)        # d at step offsets
            DBS = pool.tile([128, B, G, S], f32)       # 2^(30g) * d
            CINV = pool.tile([128, G], f32)            # 2^-30g for descale
            V = [pool.tile([128, G * K], f32, name=f"V{b}", tag=f"v{b}")
                 for b in range(B)]
            SC = [pool.tile([128, K + G * K], f32, name=f"SC{b}", tag=f"s{b}")
                  for b in range(B)]
            T1 = [pool.tile([128, G], f32, name=f"T1{b}", tag=f"t{b}")
                  for b in range(B)]
            U = [[pool.tile([128, G * K], f32, name=f"U{b}_{j}",
                            tag=f"u{b}{j}") for j in range(2)]
                 for b in range(B)]
            EB = [pool.tile([128, G, S], f32, name=f"EB{b}", tag=f"e{b}")
                  for b in range(B)]
            DEST = [pool.tile([128, G, S], f32, name=f"DEST{b}", tag=f"de{b}")
                    for b in range(B)]

            # partition p, slab fg  ->  frame f_local = fg*128 + p
            nc.sync.dma_start(
                XF[:],
                bass.AP(tensor=x_in, offset=0,
                        ap=[[HOP, 128], [HOP * 128, G], [1, FRAMELEN]]),
            )
            for b in range(B):
                nc.sync.dma_start(
                    DB[:, b, :, :],
                    bass.AP(tensor=d_in, offset=b * SPAN + WD,
                            ap=[[HOP, 128], [HOP * 128, G], [1, S]]),
                )
            for g in range(G):
                nc.vector.tensor_scalar_mul(XFS[:, g, :], XF[:, g, :], sg[g])
                nc.vector.tensor_scalar_mul(XFMUS[:, g, :], XF[:, g, :],
                                            MU * sginv[g])
                nc.vector.memset(CINV[:, g:g + 1], sginv[g])
                for b in range(B):
                    nc.vector.tensor_scalar_mul(DBS[:, b, g, :],
                                                DB[:, b, g, :], sg[g])
            nc.vector.memset(V[0][:], 0.0)
            for b in range(B):
                nc.vector.memset(SC[b][:, K - 1:K], 0.0)

            xfs0 = XFS[:, 0:1, 0:1]
            xfmus0 = XFMUS[:, 0:1, 0:1]
            aps = {b: (SC[b][:, 0:1], EB[b][:, 0:1, 0:1]) for b in range(B)}

            # Two chains (one per batch), wide over the 4 frame-groups.
            # Engine queues execute in issue order, so the emission below IS
            # the pipeline schedule: chain 1 runs half a step behind chain 0,
            # and each chain's GpSimd U-product overlaps the other chain's
            # DVE scan/clip work. T1/dif run on DVE (cheap there, and no
            # cross-engine hop after the scan).
            def emit_scan(b, t):
                nc.vector._custom_dve(
                    scandotclip, out=SC[b][:, K:K + G * K], in0=V[b][:],
                    in1=sub_ap(xfs0, t, [[FRAMELEN, G], [1, K]]),
                    s0=WMAX, s1=WMIN)

            def emit_t1dif(b, t):
                sc0 = aps[b][0]
                ret = None
                # T1 = ds_t + cum_g   (cum cols at K-1 + 64*{1..4})
                nc.gpsimd.tensor_tensor(
                    out=T1[b][:].unsqueeze(2),
                    in0=DBS[:, b, :, t:t + 1],
                    in1=sub_ap(sc0, 2 * K - 1, [[K, G], [1, 1]]),
                    op=AluOp.add)
                # dif_t = T1 - cum_{g-1}  (prev cols at K-1 + 64*{0..3})
                return nc.gpsimd.tensor_tensor(
                    out=EB[b][:, :, t:t + 1],
                    in0=T1[b][:].unsqueeze(2),
                    in1=sub_ap(sc0, K - 1, [[K, G], [1, 1]]),
                    op=AluOp.subtract)

            def emit_u(b, t):
                eb0 = aps[b][1]
                ub = U[b][t % 2]
                # U = dif_bcast * (MU * 2^-30g * x_win), split across
                # engines: groups 0-1 on GpSimd, groups 2-3 on DVE (STT).
                # The halves run in parallel, cutting the serial U stage
                # nearly in half and balancing engine load.
                H = G // 2
                nc.gpsimd.tensor_tensor(
                    out=sub_ap(ub[:, 0:1], 0, [[K, H], [1, K]]),
                    in0=sub_ap(xfmus0, t, [[FRAMELEN, H], [1, K]]),
                    in1=sub_ap(eb0, t, [[S, H], [0, K]]),
                    op=AluOp.mult)
                nc.vector.scalar_tensor_tensor(
                    out=sub_ap(ub[:, 0:1], H * K, [[K, H], [1, K]]),
                    in0=sub_ap(eb0, H * S + t, [[S, H], [0, K]]),
                    scalar=1.0,
                    in1=sub_ap(xfmus0, H * FRAMELEN + t,
                               [[FRAMELEN, H], [1, K]]),
                    op0=AluOp.mult, op1=AluOp.mult)

            def emit_clip(b, t, after=None):
                # V = clip(V) + U; `after` pins DVE order (nosync dep) so
                # the scheduler can't collapse the two chains into lockstep
                ins = nc.vector._custom_dve(
                    addclip, out=V[b][:], in0=V[b][:], in1=U[b][t % 2][:],
                    s0=WMAX, s1=WMIN)
                if after is not None:
                    from concourse.instruction_name_ordered_set import (
                        InstructionNameOrderedSet)
                    deps = InstructionNameOrderedSet()
                    deps.add(after.ins.name)
                    ins.ins.add_nosync_dependencies_from(deps)
                return ins

            # prologue: chain 0 runs its first half-step alone; V[1] is
            # zero-initialized THROUGH chain 0's first U so the scheduler
            # cannot start chain 1 until chain 0 is half a step ahead --
            # the two chains then stay in antiphase, which is what lets
            # each chain's GpSimd U overlap the other chain's DVE work.
            emit_scan(0, 0)
            dif0 = emit_t1dif(0, 0)
            emit_u(0, 0)
            nc.vector.tensor_scalar_mul(V[1][:], U[0][0][:], 0.0)
            for t in range(S):
                if t > 0:
                    emit_u(1, t - 1)
                    emit_scan(0, t)
                    dif0 = emit_t1dif(0, t)
                    emit_clip(1, t - 1, after=dif0)
                    emit_u(0, t)
                emit_scan(1, t)
                dif1 = emit_t1dif(1, t)
                emit_clip(0, t, after=dif1)
            emit_u(1, S - 1)
            emit_clip(1, S - 1)

            for b in range(B):
                # e = dif * 2^-30g ; d_est = d - e
                cb = CINV[:, 0:1]
                nc.vector.tensor_tensor(
                    out=EB[b][:], in0=EB[b][:],
                    in1=sub_ap(cb, 0, [[1, G], [0, S]]),
                    op=AluOp.mult)
                nc.vector.tensor_sub(DEST[b][:], DB[:, b, :, :], EB[b][:])

            # outputs: frames >= 1 use steps [TSTART, S); f_local = fg*128+p
            for b in range(B):
                for fg in range(G):
                    for kind, src in ((0, DEST[b][:, fg, TSTART:S]),
                                      (1, EB[b][:, fg, TSTART:S])):
                        nc.sync.dma_start(
                            bass.AP(tensor=out_main,
                                    offset=(kind * B + b) * F_LOC * TAIL
                                    + fg * 128 * TAIL,
                                    ap=[[TAIL, 128], [1, TAIL]]),
                            src,
                        )
            # head: local frame 0 = (fg=0, p=0)
            for b in range(B):
                for kind, src in ((0, DEST[b][0:1, 0, 0:TSTART]),
                                  (1, EB[b][0:1, 0, 0:TSTART])):
                    nc.sync.dma_start(
                        bass.AP(tensor=out_head,
                                offset=(kind * B + b) * TSTART,
                                ap=[[TSTART, 1], [1, TSTART]]),
                        src,
                    )
    nc.finalize()
    return nc


def _get_nc():
    if "nc" not in _CACHE:
        _CACHE["nc"] = _build()
    return _CACHE["nc"]


def run_shards(d, x, trace=False, **kw):
    from concourse.bass_utils import run_bass_kernel_spmd

    nc = _get_nc()
    in_maps = []
    for c in range(NC):
        lo = c * CORE_STRIDE
        in_maps.append({
            "x": np.ascontiguousarray(x[lo:lo + SPAN], dtype=np.float32),
            "d": np.ascontiguousarray(d[:, lo:lo + SPAN], dtype=np.float32),
        })
    return run_bass_kernel_spmd(nc, in_maps, core_ids=list(range(NC)),
                                trace=trace, **kw)


def assemble(results):
    mains = np.stack([r["out_main"] for r in results])  # (8, 2, B, 512, 192)
    head = results[0]["out_head"]                       # (2, B, 224)
    outs = []
    for kind in range(2):
        m = mains[:, kind].transpose(1, 0, 2, 3).reshape(B, F, TAIL)
        o = np.zeros((B, OUT_LEN), np.float32)
        o[:, WD:WD + TSTART] = head[kind]
        o[:, WD + TSTART:FRAMELEN - K] = m[:, 0]
        o[:, FRAMELEN - K:] = m[:, 1:].reshape(B, -1)
        outs.append(o)
    return outs[0], outs[1]


def kernel(d, x):
    res = run_shards(d, x)
    return assemble(res.results)


# revision 22
# speedup vs baseline: 1.1056x; 1.1056x over previous
"""LMS adaptive filter (BaseFilter) on 8 TRN2 NeuronCores.

Algorithm per (batch b, frame f): 64-tap LMS over 416 sequential steps.
  e_t   = d[b, 256f + 32 + t] - sum_k w[k] * x[256f + t + k]
  w     = clip(w + MU * e_t * x[256f + t : +64], +-65535)
The clip is essential: mu*|x_win|^2 ~ 3.2 > 2 makes the recursion
unstable, so w rides the clip rails and the rails keep all float
implementations shadowing each other.
Outputs (overlap-add): d_est = d - e and e, assembled per reference.

Sharding: 4096 frames split 512/core (both batches on every core) ->
1024 independent sequences/core = 2 chains (one per batch) x 4
frame-groups x 128 partitions.

Per step, per chain, ops are WIDE (all 4 groups at once, 256 elems):
  DVE   custom scan-dot-clip : SC = cumsum(-(clip(V) * xs_win)) run
        continuously across the 4 groups; group g's x is pre-scaled by
        2^(30g) (exact in fp32), which puts each group's dot in a
        disjoint exponent range so differencing the boundary columns
        recovers each per-group dot at full local precision (plain
        differencing of an unscaled cumsum loses ~1e-2 rel; scaled
        differencing measures 8e-5 in numpy vs the reference).
  GpSimd tensor_tensor x2    : dif = (d_scaled_t + cum_g) - cum_{g-1}
        (= 2^(30g) * e_t, stored to EB; descaled once at the end)
  GpSimd tensor_tensor       : U = dif_bcast * (MU * 2^-30g * x_win)
        (stride-0 broadcast AP; the descale is folded into the
        precomputed multiplier tile, so U = MU * e * x exactly)
  DVE   custom add-clip      : V = clip(V) + U   (V stored unclipped;
        clip applied inline on every read -> no separate clip pass)

Scheduling is as important as the op set: the Tile list-scheduler
otherwise collapses the two chains into lockstep (both Us serialized
on GpSimd while DVE idles). Three measures force a software-pipelined
antiphase steady state: (1) chain 1's V is zero-initialized THROUGH
chain 0's first U product, so chain 1 starts half a step behind;
(2) each add-clip carries a nosync (order-only) dependency on the
OTHER chain's just-emitted dif, which pins the per-period DVE order;
(3) U is double-buffered by step parity so GpSimd's U(t+1) write does
not wait on the clip(t) read. Steady state: GpSimd ~2.5us/step busy
(U 794ns, T1 200ns, dif 270ns per chain) vs DVE ~1.8us (customs
~424ns each), period ~2.88us/step.

Measured alternatives that LOST: scalar_tensor_tensor is rejected on
the GpSimd engine by neuronxcc; Scalar ACTIVATE (scale=e_col) costs
513ns per 64-elem group (222-cycle SBUF access); narrow per-group STT
updates (baseline-style, no U tile) cost ~200ns effective here vs
119ns in the pure-narrow baseline; putting T1/dif on DVE exposes
~300ns write-ack stalls between a chain's clip and its next scan;
dif-on-DVE alone adds GpS->DVE->GpS sem ping-pong (1707us); an
EB layout transposed to [128, S, G] for contiguous dif writes fell
off the GpSimd fast path entirely (2209us). bf16 anywhere diverges:
the clipped-unstable recursion amplifies perturbations ~1e3x.
"""

import numpy as np

HOP = 256
FRAMELEN = 512
K = 64
WD = 32
MU = 0.05
WMIN, WMAX = -65535.0, 65535.0
B = 2
F = 4096
NC = 8
F_LOC = F // NC              # 512 frames per core
G = 4                        # frame-groups per chain (1 chain per batch)
SCALE_BITS = 30              # per-group exponent separation
S = (FRAMELEN - K) - WD      # 416 sequential steps
TSTART = (FRAMELEN - HOP) - WD  # 224: first step kept for frames >= 1
TAIL = S - TSTART            # 192 output elements per frame >= 1
SPAN = HOP * (F_LOC - 1) + FRAMELEN  # 131328: x/d elements per core shard
CORE_STRIDE = HOP * F_LOC    # 131072
OUT_LEN = (FRAMELEN - K) + (F - 1) * TAIL  # 786688

_CACHE = {}


def _custom_ops():
    """Register the two fused DVE ops (runtime registration; shas computed
    live so the pinned-sha check in dve_table_for_ops passes)."""
    import concourse.dve_ops as dve_ops
    from concourse.dve_ops import DveOp
    from concourse.dve_spec import (
        Spec, Src0, Src1, C0, C1, Zero, scan, AluOp as DveAluOp,
        minn, maxx, lower, _has_src1,
    )
    from concourse.dve_uop import DveOpSpec

    def _ref_scandot_plain(in0, in1, s0, s1, imm2):
        p = in0.shape[0]
        a = in0.astype(np.float32).reshape(p, -1)
        b = in1.astype(np.float32).reshape(p, -1)
        return np.cumsum(-(a * b), axis=-1).astype(np.float32)

    def _ref_addclip2(in0, in1, s0, s1, imm2):
        v = in0.astype(np.float32) + in1.astype(np.float32)
        return np.clip(v, np.asarray(s1, np.float32).reshape(-1, 1),
                       np.asarray(s0, np.float32).reshape(-1, 1))

    def _register(name, spec, subdim=False):
        for op in dve_ops.OPS:
            if op.name == name:
                return op
        shas = {}
        for ver in ("v3", "v4"):
            tmp = DveOpSpec(name=name, opcode=1, uops=lower(spec, ver=ver),
                            rd1_en=_has_src1(spec))
            shas[ver] = tmp.sha(ver)
        op = DveOp(name, spec, subdim=subdim, uops_sha=shas)
        dve_ops.OPS.append(op)
        dve_ops.CUSTOM_DVE_SPECS[name] = spec
        dve_ops._SUB_OPCODE_FOR_NAME[name] = (
            dve_ops._CUSTOM_DVE_ROW_BASE + len(dve_ops.OPS) - 1)
        return op

    scandotclip = _register(
        "LMS_SCANDOT",
        Spec(body=scan(DveAluOp.ADD, Zero - Src0 * Src1, init=Zero),
             reference=_ref_scandot_plain),
    )
    addclip = _register(
        "LMS_ADDCLIP2",
        Spec(body=maxx(minn(Src0 + Src1, C0), C1),
             reference=_ref_addclip2),
    )
    return scandotclip, addclip


def _build():
    import concourse.bacc as bacc
    import concourse.tile as tile
    from concourse import mybir
    import concourse.bass as bass

    f32 = mybir.dt.float32
    AluOp = mybir.AluOpType
    Act = mybir.ActivationFunctionType
    scandotclip, addclip = _custom_ops()

    nc = bacc.Bacc("TRN2", target_bir_lowering=False)
    x_in = nc.dram_tensor("x", [SPAN], f32, kind="ExternalInput")
    d_in = nc.dram_tensor("d", [B, SPAN], f32, kind="ExternalInput")
    # [kind(0=d_est,1=e)][b][f_local][j] , j <-> step t = TSTART + j
    out_main = nc.dram_tensor("out_main", [2, B, F_LOC, TAIL], f32,
                              kind="ExternalOutput")
    # frame 0 of this core: steps t < TSTART   [kind][b][t]
    out_head = nc.dram_tensor("out_head", [2, B, TSTART], f32,
                              kind="ExternalOutput")

    def sub_ap(base, offset_add, free_dims):
        # raw AP into a tile: keep partition dim, replace free dims
        return bass.AP(tensor=base.tensor, offset=base.offset + offset_add,
                       ap=[base.ap[0]] + free_dims)

    sg = [float(2.0 ** (SCALE_BITS * g)) for g in range(G)]
    sginv = [float(2.0 ** (-SCALE_BITS * g)) for g in range(G)]

    with tile.TileContext(nc) as tc:
        with tc.tile_pool(name="p", bufs=1) as pool:
            XF = pool.tile([128, G, FRAMELEN], f32)    # x frames (slab fg)
            XFS = pool.tile([128, G, FRAMELEN], f32)   # 2^(30g) * x
            XFMUS = pool.tile([128, G, FRAMELEN], f32)  # MU * 2^-30g * x
            DB = pool.tile([128, B, G, S], f32)        # d at step offsets
            DBS = pool.tile([128, B, G, S], f32)       # 2^(30g) * d
            CINV = pool.tile([128, G], f32)            # 2^-30g for descale
            V = [pool.tile([128, G * K], f32, name=f"V{b}", tag=f"v{b}")
                 for b in range(B)]
            SC = [pool.tile([128, K + G * K], f32, name=f"SC{b}", tag=f"s{b}")
                  for b in range(B)]
            T1 = [pool.tile([128, G], f32, name=f"T1{b}", tag=f"t{b}")
                  for b in range(B)]
            U = [[pool.tile([128, G * K], f32, name=f"U{b}_{j}",
                            tag=f"u{b}{j}") for j in range(2)]
                 for b in range(B)]
            EB = [pool.tile([128, G, S], f32, name=f"EB{b}", tag=f"e{b}")
                  for b in range(B)]
            DEST = [pool.tile([128, G, S], f32, name=f"DEST{b}", tag=f"de{b}")
                    for b in range(B)]

            # partition p, slab fg  ->  frame f_local = fg*128 + p
            nc.sync.dma_start(
                XF[:],
                bass.AP(tensor=x_in, offset=0,
                        ap=[[HOP, 128], [HOP * 128, G], [1, FRAMELEN]]),
            )
            for b in range(B):
                nc.sync.dma_start(
                    DB[:, b, :, :],
                    bass.AP(tensor=d_in, offset=b * SPAN + WD,
                            ap=[[HOP, 128], [HOP * 128, G], [1, S]]),
                )
            for g in range(G):
                nc.vector.tensor_scalar_mul(XFS[:, g, :], XF[:, g, :], sg[g])
                nc.vector.tensor_scalar_mul(XFMUS[:, g, :], XF[:, g, :],
                                            MU * sginv[g])
                nc.vector.memset(CINV[:, g:g + 1], sginv[g])
                for b in range(B):
                    nc.vector.tensor_scalar_mul(DBS[:, b, g, :],
                                                DB[:, b, g, :], sg[g])
            nc.vector.memset(V[0][:], 0.0)
            for b in range(B):
                nc.vector.memset(SC[b][:, K - 1:K], 0.0)

            xfs0 = XFS[:, 0:1, 0:1]
            xfmus0 = XFMUS[:, 0:1, 0:1]
            aps = {b: (SC[b][:, 0:1], EB[b][:, 0:1, 0:1]) for b in range(B)}

            # Two chains (one per batch), wide over the 4 frame-groups.
            # Engine queues execute in issue order, so the emission below IS
            # the pipeline schedule: chain 1 runs half a step behind chain 0,
            # and each chain's GpSimd U-product overlaps the other chain's
            # DVE scan/clip work. T1/dif run on DVE (cheap there, and no
            # cross-engine hop after the scan).
            def emit_scan(b, t):
                nc.vector._custom_dve(
                    scandotclip, out=SC[b][:, K:K + G * K], in0=V[b][:],
                    in1=sub_ap(xfs0, t, [[FRAMELEN, G], [1, K]]),
                    s0=WMAX, s1=WMIN)

            def emit_t1dif(b, t):
                sc0 = aps[b][0]
                ret = None
                # T1 = ds_t + cum_g   (cum cols at K-1 + 64*{1..4})
                nc.gpsimd.tensor_tensor(
                    out=T1[b][:].unsqueeze(2),
                    in0=DBS[:, b, :, t:t + 1],
                    in1=sub_ap(sc0, 2 * K - 1, [[K, G], [1, 1]]),
                    op=AluOp.add)
                # dif_t = T1 - cum_{g-1}  (prev cols at K-1 + 64*{0..3})
                return nc.gpsimd.tensor_tensor(
                    out=EB[b][:, :, t:t + 1],
                    in0=T1[b][:].unsqueeze(2),
                    in1=sub_ap(sc0, K - 1, [[K, G], [1, 1]]),
                    op=AluOp.subtract)

            def emit_u(b, t):
                eb0 = aps[b][1]
                ub = U[b][t % 2]
                # U = dif_bcast * (MU * 2^-30g * x_win)
                nc.gpsimd.tensor_tensor(
                    out=sub_ap(ub[:, 0:1], 0, [[K, G], [1, K]]),
                    in0=sub_ap(xfmus0, t, [[FRAMELEN, G], [1, K]]),
                    in1=sub_ap(eb0, t, [[S, G], [0, K]]),
                    op=AluOp.mult)

            def emit_clip(b, t, after=None):
                # V = clip(V) + U; `after` pins DVE order (nosync dep) so
                # the scheduler can't collapse the two chains into lockstep
                ins = nc.vector._custom_dve(
                    addclip, out=V[b][:], in0=V[b][:], in1=U[b][t % 2][:],
                    s0=WMAX, s1=WMIN)
                if after is not None:
                    from concourse.instruction_name_ordered_set import (
                        InstructionNameOrderedSet)
                    deps = InstructionNameOrderedSet()
                    deps.add(after.ins.name)
                    ins.ins.add_nosync_dependencies_from(deps)
                return ins

            # prologue: chain 0 runs its first half-step alone; V[1] is
            # zero-initialized THROUGH chain 0's first U so the scheduler
            # cannot start chain 1 until chain 0 is half a step ahead --
            # the two chains then stay in antiphase, which is what lets
            # each chain's GpSimd U overlap the other chain's DVE work.
            emit_scan(0, 0)
            dif0 = emit_t1dif(0, 0)
            emit_u(0, 0)
            nc.vector.tensor_scalar_mul(V[1][:], U[0][0][:], 0.0)
            for t in range(S):
                if t > 0:
                    emit_scan(0, t)
                    dif0 = emit_t1dif(0, t)
                    emit_u(1, t - 1)
                    emit_clip(1, t - 1, after=dif0)
                emit_scan(1, t)
                dif1 = emit_t1dif(1, t)
                if t > 0:
                    emit_u(0, t)
                emit_clip(0, t, after=dif1)
            emit_u(1, S - 1)
            emit_clip(1, S - 1)

            for b in range(B):
                # e = dif * 2^-30g ; d_est = d - e
                cb = CINV[:, 0:1]
                nc.vector.tensor_tensor(
                    out=EB[b][:], in0=EB[b][:],
                    in1=sub_ap(cb, 0, [[1, G], [0, S]]),
                    op=AluOp.mult)
                nc.vector.tensor_sub(DEST[b][:], DB[:, b, :, :], EB[b][:])

            # outputs: frames >= 1 use steps [TSTART, S); f_local = fg*128+p
            for b in range(B):
                for fg in range(G):
                    for kind, src in ((0, DEST[b][:, fg, TSTART:S]),
                                      (1, EB[b][:, fg, TSTART:S])):
                        nc.sync.dma_start(
                            bass.AP(tensor=out_main,
                                    offset=(kind * B + b) * F_LOC * TAIL
                                    + fg * 128 * TAIL,
                                    ap=[[TAIL, 128], [1, TAIL]]),
                            src,
                        )
            # head: local frame 0 = (fg=0, p=0)
            for b in range(B):
                for kind, src in ((0, DEST[b][0:1, 0, 0:TSTART]),
                                  (1, EB[b][0:1, 0, 0:TSTART])):
                    nc.sync.dma_start(
                        bass.AP(tensor=out_head,
                                offset=(kind * B + b) * TSTART,
                                ap=[[TSTART, 1], [1, TSTART]]),
                        src,
                    )
    nc.finalize()
    return nc


def _get_nc():
    if "nc" not in _CACHE:
        _CACHE["nc"] = _build()
    return _CACHE["nc"]


def run_shards(d, x, trace=False, **kw):
    from concourse.bass_utils import run_bass_kernel_spmd

    nc = _get_nc()
    in_maps = []
    for c in range(NC):
        lo = c * CORE_STRIDE
        in_maps.append({
            "x": np.ascontiguousarray(x[lo:lo + SPAN], dtype=np.float32),
            "d": np.ascontiguousarray(d[:, lo:lo + SPAN], dtype=np.float32),
        })
    return run_bass_kernel_spmd(nc, in_maps, core_ids=list(range(NC)),
                                trace=trace, **kw)


def assemble(results):
    mains = np.stack([r["out_main"] for r in results])  # (8, 2, B, 512, 192)
    head = results[0]["out_head"]                       # (2, B, 224)
    outs = []
    for kind in range(2):
        m = mains[:, kind].transpose(1, 0, 2, 3).reshape(B, F, TAIL)
        o = np.zeros((B, OUT_LEN), np.float32)
        o[:, WD:WD + TSTART] = head[kind]
        o[:, WD + TSTART:FRAMELEN - K] = m[:, 0]
        o[:, FRAMELEN - K:] = m[:, 1:].reshape(B, -1)
        outs.append(o)
    return outs[0], outs[1]


def kernel(d, x):
    res = run_shards(d, x)
    return assemble(res.results)
